# revision 13
# baseline (speedup 1.0000x reference)
"""Trainium2 Bass kernel for loopy belief propagation (gnn_message_passing).

Distribution: directed edges sharded by destination node across 8 cores; nodes are
degree-sorted and striped into 128-row bands; each core's messages live in a layered
(jagged-diagonal) layout [128, T, 8] so segment_sum (node_sum) and the cavity
broadcast are contiguous vector ops, and node_sum needs no collective (fully local) —
only the [Q]-vector mean-field h is AllReduced.

Per BP iteration the device computes, for every directed message slot m (dst-sorted):
    X[m] = softmax(node_sum[dst_m] - h - L[m])  ==  new_psi[reverse(m)]
The fixed reverse-edge permutation (new_psi[rev(m)] = X[m]) is applied between
device launches; everything else (log1p, segment-sum, softmaxes, h) runs on device
across 8 NeuronCores per launch.
"""
import numpy as np

import concourse.bass as bass
import concourse.bacc as bacc
import concourse.mybir as mybir
import concourse.tile as tile
from concourse.bass_utils import run_bass_kernel_spmd

P = 128
NCORES = 8
Q = 8
NUM_ITER = 10

_cache = {}


# ---------------------------------------------------------------- host prep
def host_prep(edge_index, num_nodes):
    N = int(num_nodes)
    E = edge_index.shape[1]
    M = 2 * E
    dst = np.concatenate([edge_index[1], edge_index[0]]).astype(np.int64)
    mate = np.concatenate([np.arange(E, 2 * E), np.arange(0, E)]).astype(np.int64)

    deg = np.bincount(dst, minlength=N)
    order = np.argsort(-deg, kind="stable")
    rank = np.empty(N, dtype=np.int64)
    rank[order] = np.arange(N)

    NBANDS_G = ((N + NCORES * P - 1) // (NCORES * P)) * NCORES
    NB = NBANDS_G // NCORES
    NPAD = NBANDS_G * P

    deg_sorted = np.zeros(NPAD, dtype=np.int64)
    deg_sorted[:N] = deg[order]
    Kmax = max(int(deg_sorted.max()), 2)

    dg = deg_sorted.reshape(NB, NCORES, P)
    n_tilde = np.zeros(Kmax, dtype=np.int64)
    for k in range(Kmax):
        n_tilde[k] = max(int((dg >= k + 1).sum(axis=0).max()), 1)
    n_tilde[0] = NB
    off = np.zeros(Kmax + 1, dtype=np.int64)
    off[1:] = np.cumsum(n_tilde)
    T = int(off[-1])

    rank_dst = rank[dst]
    msg_order = np.argsort(rank_dst, kind="stable")
    sorted_r = rank_dst[msg_order]
    counts = deg_sorted[: int(rank_dst.max()) + 1]
    starts = np.zeros(len(counts) + 1, dtype=np.int64)
    starts[1:] = np.cumsum(counts)
    k_within = np.arange(M) - starts[sorted_r]
    g = sorted_r // P
    t_of = off[k_within] + g // NCORES

    slot_core = np.empty(M, dtype=np.int32)
    slot_p = np.empty(M, dtype=np.int32)
    slot_t = np.empty(M, dtype=np.int64)
    slot_core[msg_order] = (g % NCORES).astype(np.int32)
    slot_p[msg_order] = (sorted_r % P).astype(np.int32)
    slot_t[msg_order] = t_of

    rowmask = np.zeros((NCORES, P, T), dtype=np.float32)
    nodemask = np.zeros((NCORES, P, NB), dtype=np.float32)
    rowmask[slot_core, slot_p, slot_t] = 1.0
    rr = ((NCORES * np.arange(NB)[None, None, :] + np.arange(NCORES)[:, None, None]) * P
          + np.arange(P)[None, :, None])
    nodemask[:] = (rr < N).astype(np.float32)

    # permutation in flat slot space: new_psi_flat[slot(m)] = X_flat[slot(mate(m))]
    flat = (slot_core.astype(np.int64) * P + slot_p) * T + slot_t
    perm = np.empty(M, dtype=np.int64)
    perm[:] = flat[mate]

    return dict(T=T, NB=NB, Kmax=Kmax, n_tilde=n_tilde, off=off,
                rank=rank, slot_core=slot_core, slot_p=slot_p, slot_t=slot_t,
                flat=flat, perm=perm, rowmask=rowmask, nodemask=nodemask, deg=deg)


# ---------------------------------------------------------------- device program
def _chunks(T, n):
    bounds = [round(i * T / n) for i in range(n + 1)]
    return [(bounds[i], bounds[i + 1]) for i in range(n) if bounds[i + 1] > bounds[i]]


def build_step_program(T, NB, Kmax, n_tilde, off, final=False):
    """One BP iteration: psi (slot layout) -> X = softmax(ns - h - L).
    final=True builds the cheap epilogue instead: psi -> marginal only."""
    nc = bacc.Bacc("TRN2", target_bir_lowering=False, debug=False, num_devices=NCORES)
    f32 = mybir.dt.float32

    psi_in = nc.dram_tensor("psi_in", [P, T * Q], f32, kind="ExternalInput").ap()
    nmask_in = nc.dram_tensor("nmask_in", [P, NB], f32, kind="ExternalInput").ap()
    wcol_in = nc.dram_tensor("wcol_in", [P, 1], f32, kind="ExternalInput").ap()
    onesbn_in = nc.dram_tensor("onesbn_in", [P, P], f32, kind="ExternalInput").ap()
    if final:
        marg_out = nc.dram_tensor("marg_out", [P, NB * Q], f32, kind="ExternalOutput").ap()
    else:
        x_out = nc.dram_tensor("x_out", [P, T * Q], f32, kind="ExternalOutput").ap()
        h_in = nc.dram_tensor("h_in", [P, Q], f32).ap()
        h_out = nc.dram_tensor("h_out", [P, Q], f32, addr_space="Shared").ap()

    TT, AX, OP, AF = (nc.vector.tensor_tensor, mybir.AxisListType,
                      mybir.AluOpType, mybir.ActivationFunctionType)

    kgroups = []
    k = 0
    while k < Kmax:
        k2 = k
        while k2 + 1 < Kmax and n_tilde[k2 + 1] == n_tilde[k]:
            k2 += 1
        kgroups.append((k, k2 - k + 1, int(n_tilde[k])))
        k = k2 + 1

    NCHUNK = 4
    with tile.TileContext(nc) as tc:
        with tc.tile_pool(name="sbuf", bufs=1) as pool, \
             tc.tile_pool(name="psum", bufs=1, space="PSUM") as psum_tp:
            psi = pool.tile([P, T, Q], f32)
            work = pool.tile([P, T, Q], f32)
            red = pool.tile([P, T], f32)
            nmask = pool.tile([P, NB], f32)
            wcol = pool.tile([P, 1], f32)
            onesbn = pool.tile([P, P], f32)
            ns = pool.tile([P, NB, Q], f32)
            nsa = pool.tile([P, NB, Q], f32)
            nsb = pool.tile([P, NB, Q], f32)
            redN = pool.tile([P, NB], f32)
            hrepl = pool.tile([P, Q], f32)
            hl = pool.tile([P, Q], f32)

            nc.sync.dma_start(out=nmask[:], in_=nmask_in[:])
            nc.sync.dma_start(out=wcol[:], in_=wcol_in[:])
            nc.sync.dma_start(out=onesbn[:], in_=onesbn_in[:])
            # chunked psi load + L = ln(w*psi + 1), so ACT starts while DMA streams
            for (a, b_) in _chunks(T, NCHUNK):
                nc.sync.dma_start(
                    out=psi[:, a:b_, :].rearrange("p t q -> p (t q)"),
                    in_=psi_in[:, a * Q:b_ * Q])
                nc.scalar.activation(out=work[:, a:b_, :], in_=psi[:, a:b_, :],
                                     func=AF.Ln, scale=wcol[:, 0:1], bias=1.0)
            # node_sum via layers, split across DVE and GpSimd accumulators
            nsg = pool.tile([P, NB, Q], f32)
            kh = max(2, Kmax // 2)
            nc.vector.tensor_copy(out=ns[:, 0:int(n_tilde[0]), :],
                                  in_=work[:, int(off[0]):int(off[0] + n_tilde[0]), :])
            for kk in range(1, kh):
                nt, o = int(n_tilde[kk]), int(off[kk])
                nc.vector.tensor_add(out=ns[:, 0:nt, :], in0=ns[:, 0:nt, :],
                                     in1=work[:, o:o + nt, :])
            nt0 = int(n_tilde[kh])
            nc.gpsimd.tensor_copy(out=nsg[:, 0:nt0, :],
                                  in_=work[:, int(off[kh]):int(off[kh]) + nt0, :])
            for kk in range(kh + 1, Kmax):
                nt, o = int(n_tilde[kk]), int(off[kk])
                nc.gpsimd.tensor_add(out=nsg[:, 0:nt, :], in0=nsg[:, 0:nt, :],
                                     in1=work[:, o:o + nt, :])
            nc.vector.tensor_add(out=ns[:, 0:nt0, :], in0=ns[:, 0:nt0, :],
                                 in1=nsg[:, 0:nt0, :])
            # marginal rows: nsa = exp(ns - rowmax), redN = 1/rowsum
            nc.vector.tensor_reduce(out=redN[:], in_=ns[:], axis=AX.X, op=OP.max)
            TT(out=nsa[:], in0=ns[:],
               in1=redN[:, :, None].to_broadcast([P, NB, Q]), op=OP.subtract)
            nc.scalar.activation(out=nsa[:], in_=nsa[:], func=AF.Exp)
            nc.vector.tensor_reduce(out=redN[:], in_=nsa[:], axis=AX.X, op=OP.add)
            nc.vector.reciprocal(out=redN[:], in_=redN[:])
            if final:
                TT(out=nsb[:], in0=nsa[:],
                   in1=redN[:, :, None].to_broadcast([P, NB, Q]), op=OP.mult)
                nc.sync.dma_start(out=marg_out[:], in_=nsb[:].rearrange("p n q -> p (n q)"))
            else:
                _build_step_tail(nc, tc, pool, psum_tp, TT, AX, OP, AF,
                                 T, NB, Kmax, n_tilde, off, kgroups, NCHUNK,
                                 psi, work, red, nmask, wcol, onesbn, ns, nsa, nsb,
                                 redN, hrepl, hl, x_out, h_in, h_out)

    nc.compile()
    return nc


def _build_step_tail(nc, tc, pool, psum_tp, TT, AX, OP, AF,
                     T, NB, Kmax, n_tilde, off, kgroups, NCHUNK,
                     psi, work, red, nmask, wcol, onesbn, ns, nsa, nsb,
                     redN, hrepl, hl, x_out, h_in, h_out):
            f32 = mybir.dt.float32
            # masked marginal -> h = (b/N) * global sum (AllReduce overlaps cavity below)
            nc.vector.tensor_mul(out=redN[:], in0=redN[:], in1=nmask[:])
            TT(out=nsb[:], in0=nsa[:],
               in1=redN[:, :, None].to_broadcast([P, NB, Q]), op=OP.mult)
            cur = NB
            while cur > 1:
                half = cur // 2
                nc.vector.tensor_add(out=nsb[:, 0:half, :], in0=nsb[:, 0:half, :],
                                     in1=nsb[:, cur - half:cur, :])
                cur = cur - half
            psum = psum_tp.tile([P, Q], f32, space="PSUM")
            nc.tensor.matmul(out=psum[:], lhsT=onesbn[:], rhs=nsb[:, 0, :],
                             start=True, stop=True)
            nc.vector.tensor_copy(out=hl[:], in_=psum[:])
            nc.sync.dma_start(out=h_in[:], in_=hl[:])
            nc.gpsimd.collective_compute(
                "AllReduce", OP.add, replica_groups=[list(range(NCORES))],
                ins=[h_in[:]], outs=[h_out[:]])
            nc.sync.dma_start(out=hrepl[:], in_=h_out[:])
            # h-independent part of the cavity overlaps the AllReduce:
            # nsa2 = ns - rowmax(ns)   (reuse redN? redN holds recip; use a fresh max)
            redM = pool.tile([P, NB], f32)
            nc.vector.tensor_reduce(out=redM[:], in_=ns[:], axis=AX.X, op=OP.max)
            TT(out=nsa[:], in0=ns[:],
               in1=redM[:, :, None].to_broadcast([P, NB, Q]), op=OP.subtract)
            # work = L - nsa2  (big, h-free, overlaps AR)
            for (k0, gcnt, nt) in kgroups:
                o = int(off[k0])
                TT(out=work[:, o:o + gcnt * nt, :].rearrange("p (g n) q -> p g n q", g=gcnt),
                   in0=work[:, o:o + gcnt * nt, :].rearrange("p (g n) q -> p g n q", g=gcnt),
                   in1=nsa[:, None, 0:nt, :].to_broadcast([P, gcnt, nt, Q]),
                   op=OP.subtract)
            # u = exp(-h) broadcast factor
            u = pool.tile([P, Q], f32)
            nc.scalar.activation(out=u[:], in_=hrepl[:], func=AF.Exp, scale=-1.0)
            # X = softmax over q: E = exp(-work)*u ; X = E / rowsum(E)  (chunked + store)
            for (a, b_) in _chunks(T, NCHUNK):
                nc.scalar.activation(out=work[:, a:b_, :], in_=work[:, a:b_, :],
                                     func=AF.Exp, scale=-1.0)
                nc.gpsimd.tensor_tensor(
                    out=work[:, a:b_, :], in0=work[:, a:b_, :],
                    in1=u[:, None, :].to_broadcast([P, b_ - a, Q]), op=OP.mult)
                nc.vector.tensor_reduce(out=red[:, a:b_], in_=work[:, a:b_, :],
                                        axis=AX.X, op=OP.add)
                nc.vector.reciprocal(out=red[:, a:b_], in_=red[:, a:b_])
                TT(out=work[:, a:b_, :], in0=work[:, a:b_, :],
                   in1=red[:, a:b_, None].to_broadcast([P, b_ - a, Q]), op=OP.mult)
                nc.sync.dma_start(out=x_out[:, a * Q:b_ * Q],
                                  in_=work[:, a:b_, :].rearrange("p t q -> p (t q)"))


def _get_compiled(prep):
    key = (prep["T"], prep["NB"], prep["Kmax"], tuple(prep["n_tilde"].tolist()))
    if key not in _cache:
        step = build_step_program(prep["T"], prep["NB"], prep["Kmax"],
                                  prep["n_tilde"], prep["off"], final=False)
        fin = build_step_program(prep["T"], prep["NB"], prep["Kmax"],
                                 prep["n_tilde"], prep["off"], final=True)
        _cache[key] = (step, fin)
    return _cache[key]


# ---------------------------------------------------------------- runner
def run(edge_index, psi0, beta, num_nodes, trace=False):
    edge_index = np.asarray(edge_index)
    psi0 = np.asarray(psi0, dtype=np.float32)
    beta = np.asarray(beta, dtype=np.float32)
    N = int(num_nodes)
    prep = host_prep(edge_index, N)
    nc_step, nc_fin = _get_compiled(prep)

    T, NB = prep["T"], prep["NB"]
    sc, sp, st = prep["slot_core"], prep["slot_p"], prep["slot_t"]
    flat, perm = prep["flat"], prep["perm"]

    b = float(beta[0])
    w = np.float32(np.exp(b) - 1.0)
    wcol = np.full((P, 1), w, dtype=np.float32)
    onesbn = np.full((P, P), b / N, dtype=np.float32)

    # init: psi = softmax(psi0) in slot layout (fp32, matches reference init_bp)
    psi_flat = np.zeros((NCORES * P * T, Q), dtype=np.float32)
    m_ = psi0.max(axis=-1, keepdims=True)
    e_ = np.exp(psi0 - m_, dtype=np.float32)
    psi_flat[flat] = e_ / e_.sum(axis=-1, keepdims=True)

    base_maps = []
    for i in range(NCORES):
        base_maps.append({
            "nmask_in": prep["nodemask"][i],
            "wcol_in": wcol,
            "onesbn_in": onesbn,
        })

    total_ns = 0
    have_ns = True
    marg_all = None
    for it in range(NUM_ITER + 1):
        in_maps = []
        pv = psi_flat.reshape(NCORES, P, T * Q)
        for i in range(NCORES):
            d = dict(base_maps[i])
            d["psi_in"] = pv[i]
            in_maps.append(d)
        nc = nc_fin if it == NUM_ITER else nc_step
        res = run_bass_kernel_spmd(nc, in_maps, core_ids=list(range(NCORES)),
                                   trace=trace)
        if res.exec_time_ns is not None:
            total_ns += int(res.exec_time_ns)
        else:
            have_ns = False
        if it == NUM_ITER:
            marg_all = np.stack([res.results[i]["marg_out"].reshape(P, NB, Q)
                                 for i in range(NCORES)])
            break
        x_flat = np.concatenate(
            [res.results[i]["x_out"].reshape(P * T, Q) for i in range(NCORES)])
        # reverse-edge shuffle: psi_next[slot(m)] = X[slot(mate(m))]
        psi_flat = np.zeros_like(psi_flat)
        psi_flat[flat] = x_flat[perm]

    message_map = psi_flat[flat]
    r = prep["rank"]
    g = r // P
    marginal = marg_all[(g % NCORES), r % P, g // NCORES].astype(np.float32)
    return (message_map, marginal), (total_ns if have_ns else None)


def kernel(edge_index, psi0, beta, num_nodes):
    (message_map, marginal), _ = run(edge_index, psi0, beta, num_nodes, trace=False)
    return message_map, marginal


# revision 14
# speedup vs baseline: 1.0344x; 1.0344x over previous
"""Trainium2 Bass kernel for loopy belief propagation (gnn_message_passing).

Distribution: directed edges sharded by destination node across 8 cores; nodes are
degree-sorted and striped into 128-row bands; each core's messages live in a layered
(jagged-diagonal) layout [128, T, 8] so segment_sum (node_sum) and the cavity
broadcast are contiguous vector ops, and node_sum needs no collective (fully local) —
only the [Q]-vector mean-field h is AllReduced.

Per BP iteration the device computes, for every directed message slot m (dst-sorted):
    X[m] = softmax(node_sum[dst_m] - h - L[m])  ==  new_psi[reverse(m)]
The fixed reverse-edge permutation (new_psi[rev(m)] = X[m]) is applied between
device launches; everything else (log1p, segment-sum, softmaxes, h) runs on device
across 8 NeuronCores per launch.
"""
import numpy as np

import concourse.bass as bass
import concourse.bacc as bacc
import concourse.mybir as mybir
import concourse.tile as tile
from concourse.bass_utils import run_bass_kernel_spmd

P = 128
NCORES = 8
Q = 8
NUM_ITER = 10

_cache = {}


# ---------------------------------------------------------------- host prep
def host_prep(edge_index, num_nodes):
    N = int(num_nodes)
    E = edge_index.shape[1]
    M = 2 * E
    dst = np.concatenate([edge_index[1], edge_index[0]]).astype(np.int64)
    mate = np.concatenate([np.arange(E, 2 * E), np.arange(0, E)]).astype(np.int64)

    deg = np.bincount(dst, minlength=N)
    order = np.argsort(-deg, kind="stable")
    rank = np.empty(N, dtype=np.int64)
    rank[order] = np.arange(N)

    NBANDS_G = ((N + NCORES * P - 1) // (NCORES * P)) * NCORES
    NB = NBANDS_G // NCORES
    NPAD = NBANDS_G * P

    deg_sorted = np.zeros(NPAD, dtype=np.int64)
    deg_sorted[:N] = deg[order]
    Kmax = max(int(deg_sorted.max()), 2)

    dg = deg_sorted.reshape(NB, NCORES, P)
    n_tilde = np.zeros(Kmax, dtype=np.int64)
    for k in range(Kmax):
        n_tilde[k] = max(int((dg >= k + 1).sum(axis=0).max()), 1)
    n_tilde[0] = NB
    off = np.zeros(Kmax + 1, dtype=np.int64)
    off[1:] = np.cumsum(n_tilde)
    T = int(off[-1])

    rank_dst = rank[dst]
    msg_order = np.argsort(rank_dst, kind="stable")
    sorted_r = rank_dst[msg_order]
    counts = deg_sorted[: int(rank_dst.max()) + 1]
    starts = np.zeros(len(counts) + 1, dtype=np.int64)
    starts[1:] = np.cumsum(counts)
    k_within = np.arange(M) - starts[sorted_r]
    g = sorted_r // P
    t_of = off[k_within] + g // NCORES

    slot_core = np.empty(M, dtype=np.int32)
    slot_p = np.empty(M, dtype=np.int32)
    slot_t = np.empty(M, dtype=np.int64)
    slot_core[msg_order] = (g % NCORES).astype(np.int32)
    slot_p[msg_order] = (sorted_r % P).astype(np.int32)
    slot_t[msg_order] = t_of

    rowmask = np.zeros((NCORES, P, T), dtype=np.float32)
    nodemask = np.zeros((NCORES, P, NB), dtype=np.float32)
    rowmask[slot_core, slot_p, slot_t] = 1.0
    rr = ((NCORES * np.arange(NB)[None, None, :] + np.arange(NCORES)[:, None, None]) * P
          + np.arange(P)[None, :, None])
    nodemask[:] = (rr < N).astype(np.float32)

    # permutation in flat slot space: new_psi_flat[slot(m)] = X_flat[slot(mate(m))]
    flat = (slot_core.astype(np.int64) * P + slot_p) * T + slot_t
    perm = np.empty(M, dtype=np.int64)
    perm[:] = flat[mate]

    return dict(T=T, NB=NB, Kmax=Kmax, n_tilde=n_tilde, off=off,
                rank=rank, slot_core=slot_core, slot_p=slot_p, slot_t=slot_t,
                flat=flat, perm=perm, rowmask=rowmask, nodemask=nodemask, deg=deg)


# ---------------------------------------------------------------- device program
def _chunks(T, n):
    bounds = [round(i * T / n) for i in range(n + 1)]
    return [(bounds[i], bounds[i + 1]) for i in range(n) if bounds[i + 1] > bounds[i]]


def build_step_program(T, NB, Kmax, n_tilde, off, final=False):
    """One BP iteration: psi (slot layout) -> X = softmax(ns - h - L).
    final=True builds the cheap epilogue instead: psi -> marginal only."""
    nc = bacc.Bacc("TRN2", target_bir_lowering=False, debug=False, num_devices=NCORES)
    f32 = mybir.dt.float32

    psi_in = nc.dram_tensor("psi_in", [P, T * Q], f32, kind="ExternalInput").ap()
    nmask_in = nc.dram_tensor("nmask_in", [P, NB], f32, kind="ExternalInput").ap()
    wcol_in = nc.dram_tensor("wcol_in", [P, 1], f32, kind="ExternalInput").ap()
    onesbn_in = nc.dram_tensor("onesbn_in", [P, P], f32, kind="ExternalInput").ap()
    if final:
        marg_out = nc.dram_tensor("marg_out", [P, NB * Q], f32, kind="ExternalOutput").ap()
    else:
        x_out = nc.dram_tensor("x_out", [P, T * Q], f32, kind="ExternalOutput").ap()
        h_in = nc.dram_tensor("h_in", [P, Q], f32).ap()
        h_out = nc.dram_tensor("h_out", [P, Q], f32, addr_space="Shared").ap()

    TT, AX, OP, AF = (nc.vector.tensor_tensor, mybir.AxisListType,
                      mybir.AluOpType, mybir.ActivationFunctionType)

    kgroups = []
    k = 0
    while k < Kmax:
        k2 = k
        while k2 + 1 < Kmax and n_tilde[k2 + 1] == n_tilde[k]:
            k2 += 1
        kgroups.append((k, k2 - k + 1, int(n_tilde[k])))
        k = k2 + 1

    NCHUNK = 4
    with tile.TileContext(nc) as tc:
        with tc.tile_pool(name="sbuf", bufs=1) as pool, \
             tc.tile_pool(name="psum", bufs=1, space="PSUM") as psum_tp:
            psi = pool.tile([P, T, Q], f32)
            work = pool.tile([P, T, Q], f32)
            red = pool.tile([P, T], f32)
            nmask = pool.tile([P, NB], f32)
            wcol = pool.tile([P, 1], f32)
            onesbn = pool.tile([P, P], f32)
            ns = pool.tile([P, NB, Q], f32)
            nsa = pool.tile([P, NB, Q], f32)
            nsb = pool.tile([P, NB, Q], f32)
            redN = pool.tile([P, NB], f32)
            hrepl = pool.tile([P, Q], f32)
            hl = pool.tile([P, Q], f32)

            nc.sync.dma_start(out=nmask[:], in_=nmask_in[:])
            nc.sync.dma_start(out=wcol[:], in_=wcol_in[:])
            nc.sync.dma_start(out=onesbn[:], in_=onesbn_in[:])
            # chunked psi load + L = ln(w*psi + 1), so ACT starts while DMA streams
            for (a, b_) in _chunks(T, NCHUNK):
                nc.sync.dma_start(
                    out=psi[:, a:b_, :].rearrange("p t q -> p (t q)"),
                    in_=psi_in[:, a * Q:b_ * Q])
                nc.scalar.activation(out=work[:, a:b_, :], in_=psi[:, a:b_, :],
                                     func=AF.Ln, scale=wcol[:, 0:1], bias=1.0)
            # node_sum via layers, split across DVE and GpSimd accumulators
            nsg = pool.tile([P, NB, Q], f32)
            kh = max(2, Kmax // 2)
            nc.vector.tensor_copy(out=ns[:, 0:int(n_tilde[0]), :],
                                  in_=work[:, int(off[0]):int(off[0] + n_tilde[0]), :])
            for kk in range(1, kh):
                nt, o = int(n_tilde[kk]), int(off[kk])
                nc.vector.tensor_add(out=ns[:, 0:nt, :], in0=ns[:, 0:nt, :],
                                     in1=work[:, o:o + nt, :])
            nt0 = int(n_tilde[kh])
            nc.gpsimd.tensor_copy(out=nsg[:, 0:nt0, :],
                                  in_=work[:, int(off[kh]):int(off[kh]) + nt0, :])
            for kk in range(kh + 1, Kmax):
                nt, o = int(n_tilde[kk]), int(off[kk])
                nc.gpsimd.tensor_add(out=nsg[:, 0:nt, :], in0=nsg[:, 0:nt, :],
                                     in1=work[:, o:o + nt, :])
            nc.vector.tensor_add(out=ns[:, 0:nt0, :], in0=ns[:, 0:nt0, :],
                                 in1=nsg[:, 0:nt0, :])
            # marginal rows: nsa = exp(ns - rowmax), redN = 1/rowsum
            nc.vector.tensor_reduce(out=redN[:], in_=ns[:], axis=AX.X, op=OP.max)
            TT(out=nsa[:], in0=ns[:],
               in1=redN[:, :, None].to_broadcast([P, NB, Q]), op=OP.subtract)
            nc.scalar.activation(out=nsa[:], in_=nsa[:], func=AF.Exp)
            nc.vector.tensor_reduce(out=redN[:], in_=nsa[:], axis=AX.X, op=OP.add)
            nc.vector.reciprocal(out=redN[:], in_=redN[:])
            if final:
                TT(out=nsb[:], in0=nsa[:],
                   in1=redN[:, :, None].to_broadcast([P, NB, Q]), op=OP.mult)
                nc.sync.dma_start(out=marg_out[:], in_=nsb[:].rearrange("p n q -> p (n q)"))
            else:
                _build_step_tail(nc, tc, pool, psum_tp, TT, AX, OP, AF,
                                 T, NB, Kmax, n_tilde, off, kgroups, NCHUNK,
                                 psi, work, red, nmask, wcol, onesbn, ns, nsa, nsb,
                                 redN, hrepl, hl, x_out, h_in, h_out)

    nc.compile()
    return nc


def _build_step_tail(nc, tc, pool, psum_tp, TT, AX, OP, AF,
                     T, NB, Kmax, n_tilde, off, kgroups, NCHUNK,
                     psi, work, red, nmask, wcol, onesbn, ns, nsa, nsb,
                     redN, hrepl, hl, x_out, h_in, h_out):
            f32 = mybir.dt.float32
            # masked marginal -> h = (b/N) * global sum (AllReduce overlaps cavity below)
            nc.vector.tensor_mul(out=redN[:], in0=redN[:], in1=nmask[:])
            TT(out=nsb[:], in0=nsa[:],
               in1=redN[:, :, None].to_broadcast([P, NB, Q]), op=OP.mult)
            cur = NB
            while cur > 1:
                half = cur // 2
                nc.vector.tensor_add(out=nsb[:, 0:half, :], in0=nsb[:, 0:half, :],
                                     in1=nsb[:, cur - half:cur, :])
                cur = cur - half
            psum = psum_tp.tile([P, Q], f32, space="PSUM")
            nc.tensor.matmul(out=psum[:], lhsT=onesbn[:], rhs=nsb[:, 0, :],
                             start=True, stop=True)
            nc.vector.tensor_copy(out=hl[:], in_=psum[:])
            nc.sync.dma_start(out=h_in[:], in_=hl[:])
            nc.gpsimd.collective_compute(
                "AllReduce", OP.add, replica_groups=[list(range(NCORES))],
                ins=[h_in[:]], outs=[h_out[:]])
            nc.sync.dma_start(out=hrepl[:], in_=h_out[:])
            # h-independent part of the cavity overlaps the AllReduce:
            # nsa2 = ns - rowmax(ns)   (reuse redN? redN holds recip; use a fresh max)
            redM = pool.tile([P, NB], f32)
            nc.vector.tensor_reduce(out=redM[:], in_=ns[:], axis=AX.X, op=OP.max)
            TT(out=nsa[:], in0=ns[:],
               in1=redM[:, :, None].to_broadcast([P, NB, Q]), op=OP.subtract)
            # work = L - nsa2  (big, h-free, overlaps AR)
            for (k0, gcnt, nt) in kgroups:
                o = int(off[k0])
                TT(out=work[:, o:o + gcnt * nt, :].rearrange("p (g n) q -> p g n q", g=gcnt),
                   in0=work[:, o:o + gcnt * nt, :].rearrange("p (g n) q -> p g n q", g=gcnt),
                   in1=nsa[:, None, 0:nt, :].to_broadcast([P, gcnt, nt, Q]),
                   op=OP.subtract)
            # u = exp(-h) broadcast factor
            u = pool.tile([P, Q], f32)
            nc.scalar.activation(out=u[:], in_=hrepl[:], func=AF.Exp, scale=-1.0)
            # X = softmax over q: E = exp(-work)*u ; X = E / rowsum(E)  (chunked + store)
            for (a, b_) in _chunks(T, NCHUNK):
                nc.scalar.activation(out=work[:, a:b_, :], in_=work[:, a:b_, :],
                                     func=AF.Exp, scale=-1.0)
                TT(out=work[:, a:b_, :], in0=work[:, a:b_, :],
                   in1=u[:, None, :].to_broadcast([P, b_ - a, Q]), op=OP.mult)
                nc.vector.tensor_reduce(out=red[:, a:b_], in_=work[:, a:b_, :],
                                        axis=AX.X, op=OP.add)
                nc.vector.reciprocal(out=red[:, a:b_], in_=red[:, a:b_])
                TT(out=work[:, a:b_, :], in0=work[:, a:b_, :],
                   in1=red[:, a:b_, None].to_broadcast([P, b_ - a, Q]), op=OP.mult)
                nc.sync.dma_start(out=x_out[:, a * Q:b_ * Q],
                                  in_=work[:, a:b_, :].rearrange("p t q -> p (t q)"))


def _get_compiled(prep):
    key = (prep["T"], prep["NB"], prep["Kmax"], tuple(prep["n_tilde"].tolist()))
    if key not in _cache:
        step = build_step_program(prep["T"], prep["NB"], prep["Kmax"],
                                  prep["n_tilde"], prep["off"], final=False)
        fin = build_step_program(prep["T"], prep["NB"], prep["Kmax"],
                                 prep["n_tilde"], prep["off"], final=True)
        _cache[key] = (step, fin)
    return _cache[key]


# ---------------------------------------------------------------- runner
def run(edge_index, psi0, beta, num_nodes, trace=False):
    edge_index = np.asarray(edge_index)
    psi0 = np.asarray(psi0, dtype=np.float32)
    beta = np.asarray(beta, dtype=np.float32)
    N = int(num_nodes)
    prep = host_prep(edge_index, N)
    nc_step, nc_fin = _get_compiled(prep)

    T, NB = prep["T"], prep["NB"]
    sc, sp, st = prep["slot_core"], prep["slot_p"], prep["slot_t"]
    flat, perm = prep["flat"], prep["perm"]

    b = float(beta[0])
    w = np.float32(np.exp(b) - 1.0)
    wcol = np.full((P, 1), w, dtype=np.float32)
    onesbn = np.full((P, P), b / N, dtype=np.float32)

    # init: psi = softmax(psi0) in slot layout (fp32, matches reference init_bp)
    psi_flat = np.zeros((NCORES * P * T, Q), dtype=np.float32)
    m_ = psi0.max(axis=-1, keepdims=True)
    e_ = np.exp(psi0 - m_, dtype=np.float32)
    psi_flat[flat] = e_ / e_.sum(axis=-1, keepdims=True)

    base_maps = []
    for i in range(NCORES):
        base_maps.append({
            "nmask_in": prep["nodemask"][i],
            "wcol_in": wcol,
            "onesbn_in": onesbn,
        })

    total_ns = 0
    have_ns = True
    marg_all = None
    for it in range(NUM_ITER + 1):
        in_maps = []
        pv = psi_flat.reshape(NCORES, P, T * Q)
        for i in range(NCORES):
            d = dict(base_maps[i])
            d["psi_in"] = pv[i]
            in_maps.append(d)
        nc = nc_fin if it == NUM_ITER else nc_step
        res = run_bass_kernel_spmd(nc, in_maps, core_ids=list(range(NCORES)),
                                   trace=trace)
        if res.exec_time_ns is not None:
            total_ns += int(res.exec_time_ns)
        else:
            have_ns = False
        if it == NUM_ITER:
            marg_all = np.stack([res.results[i]["marg_out"].reshape(P, NB, Q)
                                 for i in range(NCORES)])
            break
        x_flat = np.concatenate(
            [res.results[i]["x_out"].reshape(P * T, Q) for i in range(NCORES)])
        # reverse-edge shuffle: psi_next[slot(m)] = X[slot(mate(m))]
        psi_flat = np.zeros_like(psi_flat)
        psi_flat[flat] = x_flat[perm]

    message_map = psi_flat[flat]
    r = prep["rank"]
    g = r // P
    marginal = marg_all[(g % NCORES), r % P, g // NCORES].astype(np.float32)
    return (message_map, marginal), (total_ns if have_ns else None)


def kernel(edge_index, psi0, beta, num_nodes):
    (message_map, marginal), _ = run(edge_index, psi0, beta, num_nodes, trace=False)
    return message_map, marginal
